# revision 6
# baseline (speedup 1.0000x reference)
"""Multi-head attention (B=8, N=1024, D=768, 12 heads x 64) on 8 TRN2
NeuronCores, batch-parallel (one batch element per core, no collectives).

v2: explicit tile_position packing of the small attention matmuls.
  - S^T per head is K=64: the two heads of a pair run CONCURRENTLY on
    row strips (0,0)/(64,0) of the PE array (microbench: 4ns stagger).
  - PV per head is M=64 (ones column dropped): the two heads of a pair
    run concurrently on col strips (0,0)/(0,64).
  - softmax denominators are M=1 matmuls (lhsT=ones column) quad-packed
    on col strips (0,{0,32,64,96}), accumulated over key chunks in one
    PSUM bank pre-cleared by a zero-weight dummy matmul.
  - scores for a 4-head group live in one [128, 2048] PSUM tile so the
    exp is a single wide ScalarE activation (352-cycle overhead paid
    once per 2048 columns instead of per 512).
  - RoPE: rotate_half via a +-1 permutation matmul; the cos-multiply and
    final add run on the otherwise idle GpSimd engine (SBUF-only), the
    psum-reading sin-multiply on VectorE.
  - out-projection per (oc, qc) tile accumulates all 6 contraction
    chunks in one PSUM bank; the qc=0 half is issued right after the
    qc=0 attention pass so it fills PE slack during qc=1 attention.
Everything transposed so no on-device transposes are needed (x arrives
host-transposed; output is written transposed, host untransposes).
"""
import sys

sys.path.insert(0, "/opt/trn_rl_repo")

import numpy as np
import ml_dtypes

import concourse.bass as bass
import concourse.tile as tile_mod
from concourse import mybir
from concourse.bass_utils import run_bass_kernel_spmd
from concourse.vector_clock import ScopedClock

F32 = mybir.dt.float32
BF16 = mybir.dt.bfloat16

B, N, D = 8, 1024, 768
H, DH = 12, 64
HP = H // 2          # head pairs (two heads share a 128-partition tile)
G = HP // 2          # groups of two pairs (4 heads)
KC = D // 128        # contraction chunks for the projections
RC = N // 128        # row chunks of the sequence
NK = N // 128        # key chunks
SCALE = DH ** -0.5


# --- walrus workaround: one sync-wait per instruction ---------------------
def _patched_drain_and_barrier(self, tick_clock, wait_clock):
    drain_inst = self.nc.sync.drain()
    wait_clock.add_sem_waits(
        drain_inst.ins, ScopedClock({None: tick_clock.global_clock})
    )
    si = drain_inst.ins.sync_info
    waits = list(si.on_wait or []) if si is not None else []
    if len(waits) > 1:
        drain_inst.ins.sync_info = mybir.SyncInfo(
            on_wait=waits[:1], on_update=list(si.on_update or [])
        )
        for w in waits[1:]:
            nop = self.nc.sync.nop(nofuse=True)
            nop.ins.sync_info = mybir.SyncInfo(on_wait=[w], on_update=[])
    self.nc.all_engine_barrier()
    assert self.sems is not None
    popped = self.nc._tile_sem_poison_stack.pop()
    assert popped is self._sem_poison
    self.nc.clear_and_free_semaphores(list(self.sems.allocated().values()))
    self.nc.all_engine_barrier()


tile_mod.TileContext._drain_and_barrier = _patched_drain_and_barrier


_split_counter = [0]


def split_sync_waits(nc, max_waits=1):
    """walrus rejects instructions carrying several sem waits; spill the
    excess onto engine-matched NOPs inserted directly before the offender."""
    for f in nc.m.functions:
        for bb in f.blocks:
            il = bb.instructions
            i = 0
            while i < len(il):
                inst = il[i]
                si = inst.sync_info
                waits = list(si.on_wait or []) if si is not None else []
                if len(waits) > max_waits:
                    inst.sync_info = mybir.SyncInfo(
                        on_wait=waits[:max_waits],
                        on_update=list(si.on_update or []),
                    )
                    rest = waits[max_waits:]
                    nops = []
                    for j in range(0, len(rest), max_waits):
                        _split_counter[0] += 1
                        nop = mybir.InstNoOp(
                            name=f"I-waitsplit-{_split_counter[0]}",
                            ins=[],
                            outs=[],
                            engine=inst.engine,
                        )
                        nop.sync_info = mybir.SyncInfo(
                            on_wait=rest[j : j + max_waits], on_update=[]
                        )
                        nops.append(nop)
                    for k, nop in enumerate(nops):
                        il.insert(i + k, nop)
                    i += len(nops)
                i += 1


def _bcast_rows(dram_ap, offset_elems, parts, free):
    """AP reading dram_ap[offset : offset+free] into `parts` partitions."""
    return bass.AP(
        tensor=dram_ap.tensor,
        offset=dram_ap.offset + offset_elems,
        ap=[[0, parts], [1, free]],
    )


def build_nc(with_bias=False):
    nc = bass.Bass()
    xt_d = nc.dram_tensor("xt", [D, N], BF16, kind="ExternalInput")
    wq_d = nc.dram_tensor("wq", [D, 3 * D], BF16, kind="ExternalInput")
    wo_d = nc.dram_tensor("wo", [D, D], BF16, kind="ExternalInput")
    bo_d = nc.dram_tensor("bo", [D], BF16, kind="ExternalInput")
    cos_d = nc.dram_tensor("cos2", [128, N], BF16, kind="ExternalInput")
    sin_d = nc.dram_tensor("sin2", [128, N], BF16, kind="ExternalInput")
    perm_d = nc.dram_tensor("perm", [128, 128], BF16, kind="ExternalInput")
    out_d = nc.dram_tensor("out", [D, N], F32, kind="ExternalOutput")
    import os as _os0

    _dbg = _os0.environ.get("K_DEBUG", "0") == "1"
    if _dbg:
        dbg_q = nc.dram_tensor("dbg_q", [128, HP, N], BF16, kind="ExternalOutput")
        dbg_k = nc.dram_tensor("dbg_k", [128, HP, N], BF16, kind="ExternalOutput")
        dbg_v = nc.dram_tensor("dbg_v", [128, NK, H, DH], BF16, kind="ExternalOutput")
        dbg_attn = nc.dram_tensor("dbg_attn", [128, HP, N], BF16, kind="ExternalOutput")
        dbg_recip = nc.dram_tensor("dbg_recip", [G * 2 * 4 * 512], F32, kind="ExternalOutput")

    Exp = mybir.ActivationFunctionType.Exp

    with tile_mod.TileContext(nc) as tc:
        with (
            tc.tile_pool(name="singles", bufs=1) as singles,
            tc.tile_pool(name="apool", bufs=4) as apool,
            tc.tile_pool(name="epool", bufs=2) as epool,
            tc.tile_pool(name="bpool", bufs=4) as bpool,
            tc.tile_pool(name="dpool", bufs=1, space="DRAM") as dpool,
            tc.tile_pool(name="ps_st", bufs=1, space="PSUM") as ps_st,
            tc.tile_pool(name="ps_pv", bufs=2, space="PSUM") as ps_pv,
            tc.tile_pool(name="ps_den", bufs=1, space="PSUM") as ps_den,
            tc.tile_pool(name="ps_proj", bufs=1, space="PSUM") as ps_proj,
        ):
            # ---- static inputs -------------------------------------------
            xt_sb = singles.tile([128, KC, N], BF16)
            wq_sb = singles.tile([128, KC, 3 * D], BF16)
            wo_sb = singles.tile([128, KC, D], BF16)
            for kc in range(KC):
                nc.sync.dma_start(
                    out=xt_sb[:, kc, :], in_=xt_d[kc * 128 : (kc + 1) * 128, :]
                )
                nc.sync.dma_start(
                    out=wq_sb[:, kc, :], in_=wq_d[kc * 128 : (kc + 1) * 128, :]
                )
                nc.sync.dma_start(
                    out=wo_sb[:, kc, :], in_=wo_d[kc * 128 : (kc + 1) * 128, :]
                )
            cos_sb = singles.tile([128, N], BF16)
            nc.sync.dma_start(out=cos_sb[:], in_=cos_d[:])
            sin_sb = singles.tile([128, N], BF16)
            nc.sync.dma_start(out=sin_sb[:], in_=sin_d[:])
            perm_sb = singles.tile([128, 128], BF16)
            nc.sync.dma_start(out=perm_sb[:], in_=perm_d[:])
            bo_sb = singles.tile([1, D], BF16)
            nc.sync.dma_start(
                out=bo_sb[:], in_=bo_d[:].rearrange("(o d) -> o d", o=1)
            )
            ones512 = singles.tile([1, 512], BF16)
            nc.vector.memset(ones512[:], 1.0)
            ones_col = singles.tile([128, 1], BF16)
            nc.vector.memset(ones_col[:], 1.0)
            zeros_row = singles.tile([1, 128], BF16)
            nc.vector.memset(zeros_row[:], 0.0)

            q_sb = singles.tile([128, HP, N], BF16)
            k_sb = singles.tile([128, HP, N], BF16)
            v_sb = singles.tile([128, NK, H, DH], BF16)
            attn_sb = singles.tile([128, HP, N], BF16)
            recip_d = dpool.tile([G * 2 * 4 * 512], F32)
            recip_ap = recip_d[:]

            # ---- v projection: v^T rows via [keys, 768] psum tiles -------
            # vp split 512+256 to fit the [128,512] pv-tag slots
            def v_proj(rc):
                vpA = ps_pv.tile([128, 512], F32, tag="pv", name=f"vpA{rc}")
                for kc in range(KC):
                    nc.tensor.matmul(
                        vpA[:],
                        xt_sb[:, kc, rc * 128 : (rc + 1) * 128],
                        wq_sb[:, kc, 2 * D : 2 * D + 512],
                        start=(kc == 0),
                        stop=(kc == KC - 1),
                    )
                vpB = ps_pv.tile([128, 256], F32, tag="pv", name=f"vpB{rc}")
                for kc in range(KC):
                    nc.tensor.matmul(
                        vpB[:],
                        xt_sb[:, kc, rc * 128 : (rc + 1) * 128],
                        wq_sb[:, kc, 2 * D + 512 : 3 * D],
                        start=(kc == 0),
                        stop=(kc == KC - 1),
                    )
                nc.vector.tensor_copy(
                    v_sb[:, rc, 0:8, :],
                    vpA[:].rearrange("p (h d) -> p h d", h=8),
                )
                nc.vector.tensor_copy(
                    v_sb[:, rc, 8:12, :],
                    vpB[:].rearrange("p (h d) -> p h d", h=4),
                )

            # ---- q^T / k^T projection + RoPE, one [128, 512] tile --------
            def proj_oc(oc, qc):
                # oc 0..5 -> q pair oc ; oc 6..11 -> k pair oc-6
                col0 = oc * 128 if oc < KC else D + (oc - KC) * 128
                qkp = ps_proj.tile(
                    [128, 512], F32, tag="proj", name=f"qkp{oc}_{qc}"
                )
                for kc in range(KC):
                    nc.tensor.matmul(
                        qkp[:],
                        wq_sb[:, kc, col0 : col0 + 128],
                        xt_sb[:, kc, qc * 512 : (qc + 1) * 512],
                        start=(kc == 0),
                        stop=(kc == KC - 1),
                    )
                q0 = apool.tile([128, 512], BF16, tag="q0", name=f"q0_{oc}_{qc}")
                nc.vector.tensor_copy(q0[:], qkp[:])
                rotp = ps_proj.tile(
                    [128, 512], F32, tag="proj", name=f"rotp{oc}_{qc}"
                )
                nc.tensor.matmul(
                    rotp[:], perm_sb[:], q0[:], start=True, stop=True
                )
                cslice = slice(qc * 512, (qc + 1) * 512)
                t1 = apool.tile([128, 512], BF16, tag="t1", name=f"t1_{oc}_{qc}")
                nc.vector.tensor_mul(t1[:], rotp[:], sin_sb[:, cslice])
                t2 = apool.tile([128, 512], BF16, tag="t2", name=f"t2_{oc}_{qc}")
                nc.gpsimd.tensor_mul(t2[:], q0[:], cos_sb[:, cslice])
                dst = q_sb if oc < KC else k_sb
                nc.gpsimd.tensor_add(dst[:, oc % KC, cslice], t1[:], t2[:])

            # ---- attention for one 4-head group, one 512-query chunk -----
            def attn_group(g, qc):
                pA, pB = 2 * g, 2 * g + 1
                qslc = slice(qc * 512, (qc + 1) * 512)
                den = ps_den.tile([128, 512], F32, tag="den", name=f"den{g}_{qc}")
                # dummy matmul: zero the bank, set every has_written bit so
                # the quad-packed denominator matmuls can accumulate with
                # start=False from the first key chunk on
                nc.tensor.matmul(
                    den[:], zeros_row[:], ones512[:], start=True, stop=False,
                    skip_group_check=True,
                )
                pvA = ps_pv.tile([128, 512], F32, tag="pv", name=f"pvA{g}_{qc}")
                pvB = ps_pv.tile([128, 512], F32, tag="pv", name=f"pvB{g}_{qc}")
                # pre-clear both accumulator banks (zero data + every
                # has_written bit set) so the col-packed PV matmuls can all
                # run start=False: a start=True clear from one of a pair of
                # CONCURRENT matmuls races the partner's drain (measured
                # 3.4e-2 corruption in mb_num T2)
                for pv in (pvA, pvB):
                    nc.tensor.matmul(
                        pv[:], zeros_row[:], ones512[:], start=True, stop=False,
                        skip_group_check=True,
                    )
                for kc in range(NK):
                    kslc = slice(kc * 128, (kc + 1) * 128)
                    st = ps_st.tile(
                        [128, 2048], F32, tag="st", name=f"st{g}_{qc}_{kc}"
                    )
                    for i, hp in enumerate((pA, pB)):
                        for a in range(2):
                            po = 64 * a
                            nc.tensor.matmul(
                                st[:, (2 * i + a) * 512 : (2 * i + a + 1) * 512],
                                k_sb[po : po + 64, hp, kslc],
                                q_sb[po : po + 64, hp, qslc],
                                start=True,
                                stop=True,
                                tile_position=(po, 0),
                            )
                    e = epool.tile([128, 2048], BF16, tag="e", name=f"e{g}_{qc}_{kc}")
                    nc.scalar.activation(out=e[:], in_=st[:], func=Exp, scale=SCALE)
                    for i, pv in enumerate((pvA, pvB)):
                        for a in range(2):
                            h = 4 * g + 2 * i + a
                            nc.tensor.matmul(
                                pv[64 * a : 64 * a + 64, :],
                                v_sb[:, kc, h, :],
                                e[:, (2 * i + a) * 512 : (2 * i + a + 1) * 512],
                                start=False,
                                stop=(kc == NK - 1),
                                tile_position=(0, 64 * a),
                                skip_group_check=True,
                            )
                    for j in range(4):
                        nc.tensor.matmul(
                            den[32 * j : 32 * j + 1, :],
                            ones_col[:],
                            e[:, j * 512 : (j + 1) * 512],
                            start=False,
                            stop=(kc == NK - 1),
                            tile_position=(0, 32 * j),
                            skip_group_check=True,
                        )
                # denominators -> reciprocals -> DRAM (for partition bcast)
                rec = bpool.tile([128, 512], F32, tag="rec", name=f"rec{g}_{qc}")
                for j in range(4):
                    nc.vector.reciprocal(
                        rec[32 * j : 32 * j + 1, :], den[32 * j : 32 * j + 1, :]
                    )
                base = (g * 2 + qc) * 4 * 512
                for j in range(4):
                    nc.sync.dma_start(
                        out=recip_d[:].rearrange("(r c) -> r c", c=512)[
                            (g * 2 + qc) * 4 + j : (g * 2 + qc) * 4 + j + 1, :
                        ],
                        in_=rec[32 * j : 32 * j + 1, :],
                    )
                # broadcast reciprocals & normalize straight out of PSUM
                for i, (hp, pv) in enumerate(((pA, pvA), (pB, pvB))):
                    rb = bpool.tile([128, 512], F32, tag="rb", name=f"rb{g}_{qc}_{i}")
                    nc.sync.dma_start(
                        out=rb[0:64, :],
                        in_=_bcast_rows(recip_ap, base + (2 * i) * 512, 64, 512),
                    )
                    nc.sync.dma_start(
                        out=rb[64:128, :],
                        in_=_bcast_rows(recip_ap, base + (2 * i + 1) * 512, 64, 512),
                    )
                    nc.vector.tensor_mul(attn_sb[:, hp, qslc], pv[:], rb[:])

            # ---- out-projection for one (oc, qc) tile --------------------
            def out_proj(oc, qc):
                fps = ps_proj.tile(
                    [128, 512], F32, tag="proj", name=f"fin{oc}_{qc}"
                )
                for c in range(KC):
                    nc.tensor.matmul(
                        fps[:],
                        wo_sb[:, c, oc * 128 : (oc + 1) * 128],
                        attn_sb[:, c, qc * 512 : (qc + 1) * 512],
                        start=(c == 0),
                        stop=(not with_bias and c == KC - 1),
                    )
                if with_bias:
                    nc.tensor.matmul(
                        fps[:],
                        bo_sb[0:1, oc * 128 : (oc + 1) * 128],
                        ones512[:],
                        start=False,
                        stop=True,
                    )
                fsb = bpool.tile([128, 512], F32, tag="fsb", name=f"fsb{oc}_{qc}")
                nc.vector.tensor_copy(fsb[:], fps[:])
                nc.sync.dma_start(
                    out=out_d[oc * 128 : (oc + 1) * 128, qc * 512 : (qc + 1) * 512],
                    in_=fsb[:],
                )

            # ---- schedule (issue order = scheduler priority) -------------
            for rc in range(RC):
                v_proj(rc)
            for p in (0, 1):
                for qc in (0, 1):
                    proj_oc(p, qc)       # q of pair p
                    proj_oc(KC + p, qc)  # k of pair p
            attn_group(0, 0)
            for p in (2, 3):
                for qc in (0, 1):
                    proj_oc(p, qc)
                    proj_oc(KC + p, qc)
            attn_group(1, 0)
            for p in (4, 5):
                for qc in (0, 1):
                    proj_oc(p, qc)
                    proj_oc(KC + p, qc)
            attn_group(2, 0)
            attn_group(0, 1)
            for oc in range(KC):
                out_proj(oc, 0)
            attn_group(1, 1)
            attn_group(2, 1)
            for oc in range(KC):
                out_proj(oc, 1)
            if _dbg:
                nc.sync.dma_start(out=dbg_q[:], in_=q_sb[:])
                nc.sync.dma_start(out=dbg_k[:], in_=k_sb[:])
                nc.sync.dma_start(out=dbg_v[:], in_=v_sb[:])
                nc.sync.dma_start(out=dbg_attn[:], in_=attn_sb[:])
                nc.sync.dma_start(out=dbg_recip[:], in_=recip_d[:])

    split_sync_waits(nc, max_waits=1)
    return nc


def _host_prep(x, w_qkv, w_out, b_out):
    bf = ml_dtypes.bfloat16
    inv_freq = 1.0 / (10000.0 ** (np.arange(0, DH, 2, dtype=np.float32) / DH))
    t = np.arange(N, dtype=np.float32)
    freqs = np.outer(t, inv_freq)
    emb = np.concatenate([freqs, freqs], axis=1)        # [N, DH]
    cos2 = np.tile(np.cos(emb).T.astype(np.float32), (2, 1)).astype(bf)
    sin2 = np.tile(np.sin(emb).T.astype(np.float32), (2, 1)).astype(bf)

    perm = np.zeros((128, 128), np.float32)
    for blk in range(2):
        o = blk * 64
        for m in range(32):
            perm[o + m + 32, o + m] = -1.0
        for m in range(32, 64):
            perm[o + m - 32, o + m] = 1.0
    perm = perm.astype(bf)

    xt = np.ascontiguousarray(x.transpose(0, 2, 1)).astype(bf)
    shared = {
        "wq": np.ascontiguousarray(w_qkv).astype(bf),
        "wo": np.ascontiguousarray(w_out).astype(bf),
        "bo": np.ascontiguousarray(b_out).astype(bf),
        "cos2": np.ascontiguousarray(cos2),
        "sin2": np.ascontiguousarray(sin2),
        "perm": np.ascontiguousarray(perm),
    }
    return [dict(shared, xt=np.ascontiguousarray(xt[i])) for i in range(B)]


_NC_CACHE = {}
LAST_EXEC_NS = [None]


def _run(in_maps, trace=False, with_bias=True):
    if with_bias not in _NC_CACHE:
        _NC_CACHE[with_bias] = build_nc(with_bias=with_bias)
    res = run_bass_kernel_spmd(
        _NC_CACHE[with_bias], in_maps, list(range(B)), trace=trace
    )
    LAST_EXEC_NS[0] = res.exec_time_ns
    out_t = np.stack([np.asarray(res.results[i]["out"]) for i in range(B)])
    return np.ascontiguousarray(out_t.transpose(0, 2, 1)).astype(np.float32)


def kernel(x, w_qkv, w_out, b_out, _trace=False):
    b_out = np.asarray(b_out, dtype=np.float32)
    in_maps = _host_prep(
        np.asarray(x, dtype=np.float32),
        np.asarray(w_qkv, dtype=np.float32),
        np.asarray(w_out, dtype=np.float32),
        b_out,
    )
    return _run(in_maps, trace=_trace, with_bias=bool(np.any(b_out)))


# revision 12
# speedup vs baseline: 1.1522x; 1.1522x over previous
"""Multi-head attention (B=8, N=1024, D=768, 12 heads x 64) on 8 TRN2
NeuronCores, batch-parallel (one batch element per core, no collectives).

v2: explicit tile_position packing of the small attention matmuls.
  - S^T per head is K=64: the two heads of a pair run CONCURRENTLY on
    row strips (0,0)/(64,0) of the PE array (microbench: 4ns stagger).
  - PV per head is M=64 (ones column dropped): the two heads of a pair
    run concurrently on col strips (0,0)/(0,64).
  - softmax denominators are M=1 matmuls (lhsT=ones column) quad-packed
    on col strips (0,{0,32,64,96}), accumulated over key chunks in one
    PSUM bank pre-cleared by a zero-weight dummy matmul.
  - scores for a 4-head group live in one [128, 2048] PSUM tile so the
    exp is a single wide ScalarE activation (352-cycle overhead paid
    once per 2048 columns instead of per 512).
  - RoPE: rotate_half via a +-1 permutation matmul; the cos-multiply and
    final add run on the otherwise idle GpSimd engine (SBUF-only), the
    psum-reading sin-multiply on VectorE.
  - out-projection per (oc, qc) tile accumulates all 6 contraction
    chunks in one PSUM bank; the qc=0 half is issued right after the
    qc=0 attention pass so it fills PE slack during qc=1 attention.
Everything transposed so no on-device transposes are needed (x arrives
host-transposed; output is written transposed, host untransposes).
"""
import sys

sys.path.insert(0, "/opt/trn_rl_repo")

import numpy as np
import ml_dtypes

import concourse.bass as bass
import concourse.tile as tile_mod
from concourse import mybir
from concourse.bass_utils import run_bass_kernel_spmd
from concourse.vector_clock import ScopedClock

F32 = mybir.dt.float32
BF16 = mybir.dt.bfloat16

B, N, D = 8, 1024, 768
H, DH = 12, 64
HP = H // 2          # head pairs (two heads share a 128-partition tile)
G = HP // 2          # groups of two pairs (4 heads)
KC = D // 128        # contraction chunks for the projections
RC = N // 128        # row chunks of the sequence
NK = N // 128        # key chunks
SCALE = DH ** -0.5


# --- walrus workaround: one sync-wait per instruction ---------------------
def _patched_drain_and_barrier(self, tick_clock, wait_clock):
    drain_inst = self.nc.sync.drain()
    wait_clock.add_sem_waits(
        drain_inst.ins, ScopedClock({None: tick_clock.global_clock})
    )
    si = drain_inst.ins.sync_info
    waits = list(si.on_wait or []) if si is not None else []
    if len(waits) > 1:
        drain_inst.ins.sync_info = mybir.SyncInfo(
            on_wait=waits[:1], on_update=list(si.on_update or [])
        )
        for w in waits[1:]:
            nop = self.nc.sync.nop(nofuse=True)
            nop.ins.sync_info = mybir.SyncInfo(on_wait=[w], on_update=[])
    self.nc.all_engine_barrier()
    assert self.sems is not None
    popped = self.nc._tile_sem_poison_stack.pop()
    assert popped is self._sem_poison
    self.nc.clear_and_free_semaphores(list(self.sems.allocated().values()))
    self.nc.all_engine_barrier()


tile_mod.TileContext._drain_and_barrier = _patched_drain_and_barrier


_split_counter = [0]


def split_sync_waits(nc, max_waits=1):
    """walrus rejects instructions carrying several sem waits; spill the
    excess onto engine-matched NOPs inserted directly before the offender."""
    for f in nc.m.functions:
        for bb in f.blocks:
            il = bb.instructions
            i = 0
            while i < len(il):
                inst = il[i]
                si = inst.sync_info
                waits = list(si.on_wait or []) if si is not None else []
                if len(waits) > max_waits:
                    inst.sync_info = mybir.SyncInfo(
                        on_wait=waits[:max_waits],
                        on_update=list(si.on_update or []),
                    )
                    rest = waits[max_waits:]
                    nops = []
                    for j in range(0, len(rest), max_waits):
                        _split_counter[0] += 1
                        nop = mybir.InstNoOp(
                            name=f"I-waitsplit-{_split_counter[0]}",
                            ins=[],
                            outs=[],
                            engine=inst.engine,
                        )
                        nop.sync_info = mybir.SyncInfo(
                            on_wait=rest[j : j + max_waits], on_update=[]
                        )
                        nops.append(nop)
                    for k, nop in enumerate(nops):
                        il.insert(i + k, nop)
                    i += len(nops)
                i += 1


def _bcast_rows(dram_ap, offset_elems, parts, free):
    """AP reading dram_ap[offset : offset+free] into `parts` partitions."""
    return bass.AP(
        tensor=dram_ap.tensor,
        offset=dram_ap.offset + offset_elems,
        ap=[[0, parts], [1, free]],
    )


def build_nc(with_bias=False):
    nc = bass.Bass()
    xt_d = nc.dram_tensor("xt", [D, N], BF16, kind="ExternalInput")
    wq_d = nc.dram_tensor("wq", [D, 3 * D], BF16, kind="ExternalInput")
    wo_d = nc.dram_tensor("wo", [D, D], BF16, kind="ExternalInput")
    bo_d = nc.dram_tensor("bo", [D], BF16, kind="ExternalInput")
    cos_d = nc.dram_tensor("cos2", [128, N], BF16, kind="ExternalInput")
    sin_d = nc.dram_tensor("sin2", [128, N], BF16, kind="ExternalInput")
    perm_d = nc.dram_tensor("perm", [128, 128], BF16, kind="ExternalInput")
    out_d = nc.dram_tensor("out", [D, N], F32, kind="ExternalOutput")
    import os as _os0

    _dbg = _os0.environ.get("K_DEBUG", "0") == "1"
    if _dbg:
        dbg_q = nc.dram_tensor("dbg_q", [128, HP, N], BF16, kind="ExternalOutput")
        dbg_k = nc.dram_tensor("dbg_k", [128, HP, N], BF16, kind="ExternalOutput")
        dbg_v = nc.dram_tensor("dbg_v", [128, NK, H, DH], BF16, kind="ExternalOutput")
        dbg_attn = nc.dram_tensor("dbg_attn", [128, HP, N], BF16, kind="ExternalOutput")
        dbg_recip = nc.dram_tensor("dbg_recip", [G * 2 * 128 * 512], F32, kind="ExternalOutput")

    Exp = mybir.ActivationFunctionType.Exp

    with tile_mod.TileContext(nc) as tc:
        with (
            tc.tile_pool(name="singles", bufs=1) as singles,
            tc.tile_pool(name="apool", bufs=4) as apool,
            tc.tile_pool(name="epool", bufs=3) as epool,
            tc.tile_pool(name="bpool", bufs=4) as bpool,
            tc.tile_pool(name="dpool", bufs=1, space="DRAM") as dpool,
            tc.tile_pool(name="ps_st", bufs=2, space="PSUM") as ps_st,
            tc.tile_pool(name="ps_pv", bufs=2, space="PSUM") as ps_pv,
            tc.tile_pool(name="ps_den", bufs=1, space="PSUM") as ps_den,
            tc.tile_pool(name="ps_proj", bufs=1, space="PSUM") as ps_proj,
        ):
            # ---- static inputs -------------------------------------------
            xt_sb = singles.tile([128, KC, N], BF16)
            wq_sb = singles.tile([128, KC, 3 * D], BF16)
            wo_sb = singles.tile([128, KC, D], BF16)
            for kc in range(KC):
                nc.sync.dma_start(
                    out=xt_sb[:, kc, :], in_=xt_d[kc * 128 : (kc + 1) * 128, :]
                )
                nc.sync.dma_start(
                    out=wq_sb[:, kc, :], in_=wq_d[kc * 128 : (kc + 1) * 128, :]
                )
                nc.sync.dma_start(
                    out=wo_sb[:, kc, :], in_=wo_d[kc * 128 : (kc + 1) * 128, :]
                )
            cos_sb = singles.tile([128, N], BF16)
            nc.sync.dma_start(out=cos_sb[:], in_=cos_d[:])
            sin_sb = singles.tile([128, N], BF16)
            nc.sync.dma_start(out=sin_sb[:], in_=sin_d[:])
            perm_sb = singles.tile([128, 128], BF16)
            nc.sync.dma_start(out=perm_sb[:], in_=perm_d[:])
            bo_sb = singles.tile([1, D], BF16)
            nc.sync.dma_start(
                out=bo_sb[:], in_=bo_d[:].rearrange("(o d) -> o d", o=1)
            )
            ones512 = singles.tile([1, 512], BF16)
            nc.vector.memset(ones512[:], 1.0)
            ones32 = singles.tile([128, 32], BF16)
            nc.vector.memset(ones32[:], 1.0)
            zeros_row = singles.tile([1, 128], BF16)
            nc.vector.memset(zeros_row[:], 0.0)

            q_sb = singles.tile([128, HP, N], BF16)
            k_sb = singles.tile([128, HP, N], BF16)
            v_sb = singles.tile([128, NK, H, DH], BF16)
            attn_sb = singles.tile([128, HP, N], BF16)
            recip_d = dpool.tile([G * 2 * 128 * 512], F32)
            recip_ap = recip_d[:]

            # ---- v projection: v^T rows via [keys, 768] psum tiles -------
            # vp tiles share the "proj" tag so attention's pv/den banks are
            # never gated on the v projection; the bufs=1 copy stalls leave
            # PE slack that attention waves fill by priority
            def v_proj(rc):
                vpA = ps_proj.tile([128, 512], F32, tag="proj", name=f"vpA{rc}")
                for kc in range(KC):
                    nc.tensor.matmul(
                        vpA[:],
                        xt_sb[:, kc, rc * 128 : (rc + 1) * 128],
                        wq_sb[:, kc, 2 * D : 2 * D + 512],
                        start=(kc == 0),
                        stop=(kc == KC - 1),
                    )
                vpB = ps_proj.tile([128, 256], F32, tag="proj", name=f"vpB{rc}")
                for kc in range(KC):
                    nc.tensor.matmul(
                        vpB[:],
                        xt_sb[:, kc, rc * 128 : (rc + 1) * 128],
                        wq_sb[:, kc, 2 * D + 512 : 3 * D],
                        start=(kc == 0),
                        stop=(kc == KC - 1),
                    )
                nc.vector.tensor_copy(
                    v_sb[:, rc, 0:8, :],
                    vpA[:].rearrange("p (h d) -> p h d", h=8),
                )
                nc.vector.tensor_copy(
                    v_sb[:, rc, 8:12, :],
                    vpB[:].rearrange("p (h d) -> p h d", h=4),
                )

            # ---- q^T / k^T projection + RoPE, one [128, 512] tile --------
            def proj_oc(oc, qc):
                # oc 0..5 -> q pair oc ; oc 6..11 -> k pair oc-6
                col0 = oc * 128 if oc < KC else D + (oc - KC) * 128
                qkp = ps_proj.tile(
                    [128, 512], F32, tag="proj", name=f"qkp{oc}_{qc}"
                )
                for kc in range(KC):
                    nc.tensor.matmul(
                        qkp[:],
                        wq_sb[:, kc, col0 : col0 + 128],
                        xt_sb[:, kc, qc * 512 : (qc + 1) * 512],
                        start=(kc == 0),
                        stop=(kc == KC - 1),
                    )
                q0 = apool.tile([128, 512], BF16, tag="q0", name=f"q0_{oc}_{qc}")
                nc.vector.tensor_copy(q0[:], qkp[:])
                rotp = ps_proj.tile(
                    [128, 512], F32, tag="proj", name=f"rotp{oc}_{qc}"
                )
                nc.tensor.matmul(
                    rotp[:], perm_sb[:], q0[:], start=True, stop=True
                )
                cslice = slice(qc * 512, (qc + 1) * 512)
                t1 = apool.tile([128, 512], BF16, tag="t1", name=f"t1_{oc}_{qc}")
                nc.vector.tensor_mul(t1[:], rotp[:], sin_sb[:, cslice])
                t2 = apool.tile([128, 512], BF16, tag="t2", name=f"t2_{oc}_{qc}")
                nc.gpsimd.tensor_mul(t2[:], q0[:], cos_sb[:, cslice])
                dst = q_sb if oc < KC else k_sb
                nc.gpsimd.tensor_add(dst[:, oc % KC, cslice], t1[:], t2[:])

            # ---- attention for one 4-head group, one 512-query chunk -----
            def attn_group(g, qc):
                pA, pB = 2 * g, 2 * g + 1
                qslc = slice(qc * 512, (qc + 1) * 512)
                den = ps_den.tile([128, 512], F32, tag="den", name=f"den{g}_{qc}")
                # dummy matmul: zero the bank, set every has_written bit so
                # the quad-packed denominator matmuls can accumulate with
                # start=False from the first key chunk on (a start=True on
                # one of a CONCURRENT pack races the partner's drain)
                nc.tensor.matmul(
                    den[:], zeros_row[:], ones512[:], start=True, stop=False,
                    skip_group_check=True,
                )
                pvA = ps_pv.tile([128, 512], F32, tag="pv", name=f"pvA{g}_{qc}")
                pvB = ps_pv.tile([128, 512], F32, tag="pv", name=f"pvB{g}_{qc}")
                for pv in (pvA, pvB):
                    nc.tensor.matmul(
                        pv[:], zeros_row[:], ones512[:], start=True, stop=False,
                        skip_group_check=True,
                    )
                for kc in range(NK):
                    kslc = slice(kc * 128, (kc + 1) * 128)
                    es = []
                    for i, hp in enumerate((pA, pB)):
                        st = ps_st.tile(
                            [128, 1024], F32, tag="st", name=f"st{g}_{qc}_{kc}_{i}"
                        )
                        for a in range(2):
                            po = 64 * a
                            nc.tensor.matmul(
                                st[:, a * 512 : (a + 1) * 512],
                                k_sb[po : po + 64, hp, kslc],
                                q_sb[po : po + 64, hp, qslc],
                                start=True,
                                stop=True,
                                tile_position=(po, 0),
                            )
                        e = epool.tile(
                            [128, 1024], BF16, tag="e", name=f"e{g}_{qc}_{kc}_{i}"
                        )
                        nc.scalar.activation(
                            out=e[:], in_=st[:], func=Exp, scale=SCALE
                        )
                        es.append(e)
                        pv = (pvA, pvB)[i]
                        for a in range(2):
                            h = 4 * g + 2 * i + a
                            nc.tensor.matmul(
                                pv[64 * a : 64 * a + 64, :],
                                v_sb[:, kc, h, :],
                                e[:, a * 512 : (a + 1) * 512],
                                start=False,
                                stop=(kc == NK - 1),
                                tile_position=(0, 64 * a),
                                skip_group_check=True,
                            )
                    # denominators: M=32 (32 identical rows per head) so the
                    # reciprocal runs full-width on DVE; quad-packed
                    for j in range(4):
                        nc.tensor.matmul(
                            den[32 * j : 32 * j + 32, :],
                            ones32[:],
                            es[j // 2][:, (j % 2) * 512 : (j % 2 + 1) * 512],
                            start=False,
                            stop=(kc == NK - 1),
                            tile_position=(0, 32 * j),
                            skip_group_check=True,
                        )
                # denominators -> reciprocals -> DRAM (for partition bcast)
                rec = bpool.tile([128, 512], F32, tag="rec", name=f"rec{g}_{qc}")
                nc.vector.reciprocal(rec[:], den[:])
                base = (g * 2 + qc) * 128 * 512
                nc.sync.dma_start(
                    out=recip_d[:].rearrange("(r c) -> r c", c=512)[
                        (g * 2 + qc) * 128 : (g * 2 + qc + 1) * 128, :
                    ],
                    in_=rec[:],
                )
                # broadcast reciprocals & normalize straight out of PSUM
                for i, (hp, pv) in enumerate(((pA, pvA), (pB, pvB))):
                    rb = bpool.tile([128, 512], F32, tag="rb", name=f"rb{g}_{qc}_{i}")
                    nc.sync.dma_start(
                        out=rb[0:64, :],
                        in_=_bcast_rows(recip_ap, base + (64 * i) * 512, 64, 512),
                    )
                    nc.sync.dma_start(
                        out=rb[64:128, :],
                        in_=_bcast_rows(
                            recip_ap, base + (64 * i + 32) * 512, 64, 512
                        ),
                    )
                    nc.vector.tensor_mul(attn_sb[:, hp, qslc], pv[:], rb[:])

            # ---- out-projection for one (oc, qc) tile --------------------
            def out_proj(oc, qc):
                fps = ps_proj.tile(
                    [128, 512], F32, tag="proj", name=f"fin{oc}_{qc}"
                )
                for c in range(KC):
                    nc.tensor.matmul(
                        fps[:],
                        wo_sb[:, c, oc * 128 : (oc + 1) * 128],
                        attn_sb[:, c, qc * 512 : (qc + 1) * 512],
                        start=(c == 0),
                        stop=(not with_bias and c == KC - 1),
                    )
                if with_bias:
                    nc.tensor.matmul(
                        fps[:],
                        bo_sb[0:1, oc * 128 : (oc + 1) * 128],
                        ones512[:],
                        start=False,
                        stop=True,
                    )
                fsb = bpool.tile([128, 512], F32, tag="fsb", name=f"fsb{oc}_{qc}")
                nc.vector.tensor_copy(fsb[:], fps[:])
                nc.sync.dma_start(
                    out=out_d[oc * 128 : (oc + 1) * 128, qc * 512 : (qc + 1) * 512],
                    in_=fsb[:],
                )

            # ---- schedule (issue order = scheduler priority) -------------
            for rc in range(RC):
                v_proj(rc)
            for p in (0, 1):
                for qc in (0, 1):
                    proj_oc(p, qc)       # q of pair p
                    proj_oc(KC + p, qc)  # k of pair p
            attn_group(0, 0)
            for p in (2, 3):
                for qc in (0, 1):
                    proj_oc(p, qc)
                    proj_oc(KC + p, qc)
            attn_group(1, 0)
            for p in (4, 5):
                for qc in (0, 1):
                    proj_oc(p, qc)
                    proj_oc(KC + p, qc)
            attn_group(2, 0)
            attn_group(0, 1)
            for oc in range(KC):
                out_proj(oc, 0)
            attn_group(1, 1)
            attn_group(2, 1)
            for oc in range(KC):
                out_proj(oc, 1)
            if _dbg:
                nc.sync.dma_start(out=dbg_q[:], in_=q_sb[:])
                nc.sync.dma_start(out=dbg_k[:], in_=k_sb[:])
                nc.sync.dma_start(out=dbg_v[:], in_=v_sb[:])
                nc.sync.dma_start(out=dbg_attn[:], in_=attn_sb[:])
                nc.sync.dma_start(out=dbg_recip[:], in_=recip_d[:])

    split_sync_waits(nc, max_waits=1)
    return nc


def _host_prep(x, w_qkv, w_out, b_out):
    bf = ml_dtypes.bfloat16
    inv_freq = 1.0 / (10000.0 ** (np.arange(0, DH, 2, dtype=np.float32) / DH))
    t = np.arange(N, dtype=np.float32)
    freqs = np.outer(t, inv_freq)
    emb = np.concatenate([freqs, freqs], axis=1)        # [N, DH]
    cos2 = np.tile(np.cos(emb).T.astype(np.float32), (2, 1)).astype(bf)
    sin2 = np.tile(np.sin(emb).T.astype(np.float32), (2, 1)).astype(bf)

    perm = np.zeros((128, 128), np.float32)
    for blk in range(2):
        o = blk * 64
        for m in range(32):
            perm[o + m + 32, o + m] = -1.0
        for m in range(32, 64):
            perm[o + m - 32, o + m] = 1.0
    perm = perm.astype(bf)

    xt = np.ascontiguousarray(x.transpose(0, 2, 1)).astype(bf)
    shared = {
        "wq": np.ascontiguousarray(w_qkv).astype(bf),
        "wo": np.ascontiguousarray(w_out).astype(bf),
        "bo": np.ascontiguousarray(b_out).astype(bf),
        "cos2": np.ascontiguousarray(cos2),
        "sin2": np.ascontiguousarray(sin2),
        "perm": np.ascontiguousarray(perm),
    }
    return [dict(shared, xt=np.ascontiguousarray(xt[i])) for i in range(B)]


_NC_CACHE = {}
LAST_EXEC_NS = [None]


def _run(in_maps, trace=False, with_bias=True):
    if with_bias not in _NC_CACHE:
        _NC_CACHE[with_bias] = build_nc(with_bias=with_bias)
    res = run_bass_kernel_spmd(
        _NC_CACHE[with_bias], in_maps, list(range(B)), trace=trace
    )
    LAST_EXEC_NS[0] = res.exec_time_ns
    out_t = np.stack([np.asarray(res.results[i]["out"]) for i in range(B)])
    return np.ascontiguousarray(out_t.transpose(0, 2, 1)).astype(np.float32)


def kernel(x, w_qkv, w_out, b_out, _trace=False):
    b_out = np.asarray(b_out, dtype=np.float32)
    in_maps = _host_prep(
        np.asarray(x, dtype=np.float32),
        np.asarray(w_qkv, dtype=np.float32),
        np.asarray(w_out, dtype=np.float32),
        b_out,
    )
    return _run(in_maps, trace=_trace, with_bias=bool(np.any(b_out)))


# revision 14
# speedup vs baseline: 1.1644x; 1.0105x over previous
"""Multi-head attention (B=8, N=1024, D=768, 12 heads x 64) on 8 TRN2
NeuronCores, batch-parallel (one batch element per core, no collectives).

v2: explicit tile_position packing of the small attention matmuls.
  - S^T per head is K=64: the two heads of a pair run CONCURRENTLY on
    row strips (0,0)/(64,0) of the PE array (microbench: 4ns stagger).
  - PV per head is M=64 (ones column dropped): the two heads of a pair
    run concurrently on col strips (0,0)/(0,64).
  - softmax denominators are M=1 matmuls (lhsT=ones column) quad-packed
    on col strips (0,{0,32,64,96}), accumulated over key chunks in one
    PSUM bank pre-cleared by a zero-weight dummy matmul.
  - scores for a 4-head group live in one [128, 2048] PSUM tile so the
    exp is a single wide ScalarE activation (352-cycle overhead paid
    once per 2048 columns instead of per 512).
  - RoPE: rotate_half via a +-1 permutation matmul; the cos-multiply and
    final add run on the otherwise idle GpSimd engine (SBUF-only), the
    psum-reading sin-multiply on VectorE.
  - out-projection per (oc, qc) tile accumulates all 6 contraction
    chunks in one PSUM bank; the qc=0 half is issued right after the
    qc=0 attention pass so it fills PE slack during qc=1 attention.
Everything transposed so no on-device transposes are needed (x arrives
host-transposed; output is written transposed, host untransposes).
"""
import sys

sys.path.insert(0, "/opt/trn_rl_repo")

import numpy as np
import ml_dtypes

import concourse.bass as bass
import concourse.tile as tile_mod
from concourse import mybir
from concourse.bass_utils import run_bass_kernel_spmd
from concourse.vector_clock import ScopedClock

F32 = mybir.dt.float32
BF16 = mybir.dt.bfloat16

B, N, D = 8, 1024, 768
H, DH = 12, 64
HP = H // 2          # head pairs (two heads share a 128-partition tile)
G = HP // 2          # groups of two pairs (4 heads)
KC = D // 128        # contraction chunks for the projections
RC = N // 128        # row chunks of the sequence
NK = N // 128        # key chunks
SCALE = DH ** -0.5


# --- walrus workaround: one sync-wait per instruction ---------------------
def _patched_drain_and_barrier(self, tick_clock, wait_clock):
    drain_inst = self.nc.sync.drain()
    wait_clock.add_sem_waits(
        drain_inst.ins, ScopedClock({None: tick_clock.global_clock})
    )
    si = drain_inst.ins.sync_info
    waits = list(si.on_wait or []) if si is not None else []
    if len(waits) > 1:
        drain_inst.ins.sync_info = mybir.SyncInfo(
            on_wait=waits[:1], on_update=list(si.on_update or [])
        )
        for w in waits[1:]:
            nop = self.nc.sync.nop(nofuse=True)
            nop.ins.sync_info = mybir.SyncInfo(on_wait=[w], on_update=[])
    self.nc.all_engine_barrier()
    assert self.sems is not None
    popped = self.nc._tile_sem_poison_stack.pop()
    assert popped is self._sem_poison
    self.nc.clear_and_free_semaphores(list(self.sems.allocated().values()))
    self.nc.all_engine_barrier()


tile_mod.TileContext._drain_and_barrier = _patched_drain_and_barrier


_split_counter = [0]


def split_sync_waits(nc, max_waits=1):
    """walrus rejects instructions carrying several sem waits; spill the
    excess onto engine-matched NOPs inserted directly before the offender."""
    for f in nc.m.functions:
        for bb in f.blocks:
            il = bb.instructions
            i = 0
            while i < len(il):
                inst = il[i]
                si = inst.sync_info
                waits = list(si.on_wait or []) if si is not None else []
                if len(waits) > max_waits:
                    inst.sync_info = mybir.SyncInfo(
                        on_wait=waits[:max_waits],
                        on_update=list(si.on_update or []),
                    )
                    rest = waits[max_waits:]
                    nops = []
                    for j in range(0, len(rest), max_waits):
                        _split_counter[0] += 1
                        nop = mybir.InstNoOp(
                            name=f"I-waitsplit-{_split_counter[0]}",
                            ins=[],
                            outs=[],
                            engine=inst.engine,
                        )
                        nop.sync_info = mybir.SyncInfo(
                            on_wait=rest[j : j + max_waits], on_update=[]
                        )
                        nops.append(nop)
                    for k, nop in enumerate(nops):
                        il.insert(i + k, nop)
                    i += len(nops)
                i += 1


def _bcast_rows(dram_ap, offset_elems, parts, free):
    """AP reading dram_ap[offset : offset+free] into `parts` partitions."""
    return bass.AP(
        tensor=dram_ap.tensor,
        offset=dram_ap.offset + offset_elems,
        ap=[[0, parts], [1, free]],
    )


def build_nc(with_bias=False):
    nc = bass.Bass()
    xt_d = nc.dram_tensor("xt", [D, N], BF16, kind="ExternalInput")
    wq_d = nc.dram_tensor("wq", [D, 3 * D], BF16, kind="ExternalInput")
    wo_d = nc.dram_tensor("wo", [D, D], BF16, kind="ExternalInput")
    bo_d = nc.dram_tensor("bo", [D], BF16, kind="ExternalInput")
    cos_d = nc.dram_tensor("cos2", [128, N], BF16, kind="ExternalInput")
    sin_d = nc.dram_tensor("sin2", [128, N], BF16, kind="ExternalInput")
    perm_d = nc.dram_tensor("perm", [128, 128], BF16, kind="ExternalInput")
    out_d = nc.dram_tensor("out", [D, N], F32, kind="ExternalOutput")
    import os as _os0

    _dbg = _os0.environ.get("K_DEBUG", "0") == "1"
    if _dbg:
        dbg_q = nc.dram_tensor("dbg_q", [128, HP, N], BF16, kind="ExternalOutput")
        dbg_k = nc.dram_tensor("dbg_k", [128, HP, N], BF16, kind="ExternalOutput")
        dbg_v = nc.dram_tensor("dbg_v", [128, NK, H, DH], BF16, kind="ExternalOutput")
        dbg_attn = nc.dram_tensor("dbg_attn", [128, HP, N], BF16, kind="ExternalOutput")
        dbg_recip = nc.dram_tensor("dbg_recip", [G * 2 * 128 * 512], F32, kind="ExternalOutput")

    Exp = mybir.ActivationFunctionType.Exp

    with tile_mod.TileContext(nc) as tc:
        with (
            tc.tile_pool(name="singles", bufs=1) as singles,
            tc.tile_pool(name="apool", bufs=4) as apool,
            tc.tile_pool(name="epool", bufs=3) as epool,
            tc.tile_pool(name="bpool", bufs=4) as bpool,
            tc.tile_pool(name="dpool", bufs=1, space="DRAM") as dpool,
            tc.tile_pool(name="ps_st", bufs=2, space="PSUM") as ps_st,
            tc.tile_pool(name="ps_pv", bufs=2, space="PSUM") as ps_pv,
            tc.tile_pool(name="ps_den", bufs=1, space="PSUM") as ps_den,
            tc.tile_pool(name="ps_proj", bufs=1, space="PSUM") as ps_proj,
        ):
            # ---- static inputs -------------------------------------------
            xt_sb = singles.tile([128, KC, N], BF16)
            wq_sb = singles.tile([128, KC, 3 * D], BF16)
            wo_sb = singles.tile([128, KC, D], BF16)
            for kc in range(KC):
                nc.sync.dma_start(
                    out=xt_sb[:, kc, :], in_=xt_d[kc * 128 : (kc + 1) * 128, :]
                )
                nc.sync.dma_start(
                    out=wq_sb[:, kc, :], in_=wq_d[kc * 128 : (kc + 1) * 128, :]
                )
                nc.sync.dma_start(
                    out=wo_sb[:, kc, :], in_=wo_d[kc * 128 : (kc + 1) * 128, :]
                )
            cos_sb = singles.tile([128, N], BF16)
            nc.sync.dma_start(out=cos_sb[:], in_=cos_d[:])
            sin_sb = singles.tile([128, N], BF16)
            nc.sync.dma_start(out=sin_sb[:], in_=sin_d[:])
            perm_sb = singles.tile([128, 128], BF16)
            nc.sync.dma_start(out=perm_sb[:], in_=perm_d[:])
            bo_sb = singles.tile([1, D], BF16)
            nc.sync.dma_start(
                out=bo_sb[:], in_=bo_d[:].rearrange("(o d) -> o d", o=1)
            )
            ones512 = singles.tile([1, 512], BF16)
            nc.vector.memset(ones512[:], 1.0)
            ones32 = singles.tile([128, 32], BF16)
            nc.vector.memset(ones32[:], 1.0)
            zeros_row = singles.tile([1, 128], BF16)
            nc.vector.memset(zeros_row[:], 0.0)

            q_sb = singles.tile([128, HP, N], BF16)
            k_sb = singles.tile([128, HP, N], BF16)
            v_sb = singles.tile([128, NK, H, DH], BF16)
            attn_sb = singles.tile([128, HP, N], BF16)
            recip_d = dpool.tile([G * 2 * 128 * 512], F32)
            recip_ap = recip_d[:]

            # ---- v projection: v^T rows via [keys, 768] psum tiles -------
            # vp tiles share the "proj" tag so attention's pv/den banks are
            # never gated on the v projection; the bufs=1 copy stalls leave
            # PE slack that attention waves fill by priority
            def v_proj(rc):
                vpA = ps_proj.tile([128, 512], F32, tag="proj", name=f"vpA{rc}")
                for kc in range(KC):
                    nc.tensor.matmul(
                        vpA[:],
                        xt_sb[:, kc, rc * 128 : (rc + 1) * 128],
                        wq_sb[:, kc, 2 * D : 2 * D + 512],
                        start=(kc == 0),
                        stop=(kc == KC - 1),
                    )
                vpB = ps_proj.tile([128, 256], F32, tag="proj", name=f"vpB{rc}")
                for kc in range(KC):
                    nc.tensor.matmul(
                        vpB[:],
                        xt_sb[:, kc, rc * 128 : (rc + 1) * 128],
                        wq_sb[:, kc, 2 * D + 512 : 3 * D],
                        start=(kc == 0),
                        stop=(kc == KC - 1),
                    )
                nc.vector.tensor_copy(
                    v_sb[:, rc, 0:8, :],
                    vpA[:].rearrange("p (h d) -> p h d", h=8),
                )
                nc.vector.tensor_copy(
                    v_sb[:, rc, 8:12, :],
                    vpB[:].rearrange("p (h d) -> p h d", h=4),
                )

            # ---- q^T / k^T projection + RoPE, one [128, 512] tile --------
            def proj_oc(oc, qc):
                # oc 0..5 -> q pair oc ; oc 6..11 -> k pair oc-6
                col0 = oc * 128 if oc < KC else D + (oc - KC) * 128
                qkp = ps_proj.tile(
                    [128, 512], F32, tag="proj", name=f"qkp{oc}_{qc}"
                )
                for kc in range(KC):
                    nc.tensor.matmul(
                        qkp[:],
                        wq_sb[:, kc, col0 : col0 + 128],
                        xt_sb[:, kc, qc * 512 : (qc + 1) * 512],
                        start=(kc == 0),
                        stop=(kc == KC - 1),
                    )
                q0 = apool.tile([128, 512], BF16, tag="q0", name=f"q0_{oc}_{qc}")
                nc.vector.tensor_copy(q0[:], qkp[:])
                rotp = ps_proj.tile(
                    [128, 512], F32, tag="proj", name=f"rotp{oc}_{qc}"
                )
                nc.tensor.matmul(
                    rotp[:], perm_sb[:], q0[:], start=True, stop=True
                )
                cslice = slice(qc * 512, (qc + 1) * 512)
                t1 = apool.tile([128, 512], BF16, tag="t1", name=f"t1_{oc}_{qc}")
                nc.vector.tensor_mul(t1[:], rotp[:], sin_sb[:, cslice])
                t2 = apool.tile([128, 512], BF16, tag="t2", name=f"t2_{oc}_{qc}")
                nc.gpsimd.tensor_mul(t2[:], q0[:], cos_sb[:, cslice])
                dst = q_sb if oc < KC else k_sb
                nc.gpsimd.tensor_add(dst[:, oc % KC, cslice], t1[:], t2[:])

            # ---- attention for one 4-head group, one 512-query chunk -----
            def attn_group(g, qc):
                pA, pB = 2 * g, 2 * g + 1
                qslc = slice(qc * 512, (qc + 1) * 512)
                den = ps_den.tile([128, 512], F32, tag="den", name=f"den{g}_{qc}")
                # dummy matmul: zero the bank, set every has_written bit so
                # the quad-packed denominator matmuls can accumulate with
                # start=False from the first key chunk on (a start=True on
                # one of a CONCURRENT pack races the partner's drain)
                nc.tensor.matmul(
                    den[:], zeros_row[:], ones512[:], start=True, stop=False,
                    skip_group_check=True,
                )
                pvA = ps_pv.tile([128, 512], F32, tag="pv", name=f"pvA{g}_{qc}")
                pvB = ps_pv.tile([128, 512], F32, tag="pv", name=f"pvB{g}_{qc}")
                for pv in (pvA, pvB):
                    nc.tensor.matmul(
                        pv[:], zeros_row[:], ones512[:], start=True, stop=False,
                        skip_group_check=True,
                    )
                for kc in range(NK):
                    kslc = slice(kc * 128, (kc + 1) * 128)
                    es = []
                    for i, hp in enumerate((pA, pB)):
                        st = ps_st.tile(
                            [128, 1024], F32, tag="st", name=f"st{g}_{qc}_{kc}_{i}"
                        )
                        # high priority: the S waves feed ScalarE (the
                        # near-saturated engine); make PE prefer them over
                        # same-ready PV/den work of the previous key chunk
                        with tc.high_priority(offset=30):
                            for a in range(2):
                                po = 64 * a
                                nc.tensor.matmul(
                                    st[:, a * 512 : (a + 1) * 512],
                                    k_sb[po : po + 64, hp, kslc],
                                    q_sb[po : po + 64, hp, qslc],
                                    start=True,
                                    stop=True,
                                    tile_position=(po, 0),
                                )
                        e = epool.tile(
                            [128, 1024], BF16, tag="e", name=f"e{g}_{qc}_{kc}_{i}"
                        )
                        nc.scalar.activation(
                            out=e[:], in_=st[:], func=Exp, scale=SCALE
                        )
                        es.append(e)
                        pv = (pvA, pvB)[i]
                        for a in range(2):
                            h = 4 * g + 2 * i + a
                            nc.tensor.matmul(
                                pv[64 * a : 64 * a + 64, :],
                                v_sb[:, kc, h, :],
                                e[:, a * 512 : (a + 1) * 512],
                                start=False,
                                stop=(kc == NK - 1),
                                tile_position=(0, 64 * a),
                                skip_group_check=True,
                            )
                    # denominators: M=32 (32 identical rows per head) so the
                    # reciprocal runs full-width on DVE; quad-packed
                    for j in range(4):
                        nc.tensor.matmul(
                            den[32 * j : 32 * j + 32, :],
                            ones32[:],
                            es[j // 2][:, (j % 2) * 512 : (j % 2 + 1) * 512],
                            start=False,
                            stop=(kc == NK - 1),
                            tile_position=(0, 32 * j),
                            skip_group_check=True,
                        )
                # denominators -> reciprocals -> DRAM (for partition bcast)
                rec = bpool.tile([128, 512], F32, tag="rec", name=f"rec{g}_{qc}")
                nc.vector.reciprocal(rec[:], den[:])
                base = (g * 2 + qc) * 128 * 512
                nc.sync.dma_start(
                    out=recip_d[:].rearrange("(r c) -> r c", c=512)[
                        (g * 2 + qc) * 128 : (g * 2 + qc + 1) * 128, :
                    ],
                    in_=rec[:],
                )
                # broadcast reciprocals & normalize straight out of PSUM
                for i, (hp, pv) in enumerate(((pA, pvA), (pB, pvB))):
                    rb = bpool.tile([128, 512], F32, tag="rb", name=f"rb{g}_{qc}_{i}")
                    nc.sync.dma_start(
                        out=rb[0:64, :],
                        in_=_bcast_rows(recip_ap, base + (64 * i) * 512, 64, 512),
                    )
                    nc.sync.dma_start(
                        out=rb[64:128, :],
                        in_=_bcast_rows(
                            recip_ap, base + (64 * i + 32) * 512, 64, 512
                        ),
                    )
                    nc.vector.tensor_mul(attn_sb[:, hp, qslc], pv[:], rb[:])

            # ---- out-projection for one (oc, qc) tile --------------------
            # epilogue tiles (qc=1) alternate between the freed proj and den
            # banks (two chains in parallel) and copy out on the then-idle
            # ScalarE; qc=0 tiles run during qc=1 attention, so they stay on
            # the proj bank with DVE copies (ScalarE is exp-saturated there)
            def out_proj(oc, qc):
                pool, tag = (
                    (ps_den, "den") if (qc == 1 and oc % 2 == 1) else (ps_proj, "proj")
                )
                fps = pool.tile([128, 512], F32, tag=tag, name=f"fin{oc}_{qc}")
                for c in range(KC):
                    nc.tensor.matmul(
                        fps[:],
                        wo_sb[:, c, oc * 128 : (oc + 1) * 128],
                        attn_sb[:, c, qc * 512 : (qc + 1) * 512],
                        start=(c == 0),
                        stop=(not with_bias and c == KC - 1),
                        skip_group_check=True,
                    )
                if with_bias:
                    nc.tensor.matmul(
                        fps[:],
                        bo_sb[0:1, oc * 128 : (oc + 1) * 128],
                        ones512[:],
                        start=False,
                        stop=True,
                    )
                fsb = bpool.tile([128, 512], F32, tag="fsb", name=f"fsb{oc}_{qc}")
                if qc == 1:
                    nc.scalar.activation(
                        out=fsb[:], in_=fps[:],
                        func=mybir.ActivationFunctionType.Copy, scale=1.0,
                    )
                else:
                    nc.vector.tensor_copy(fsb[:], fps[:])
                nc.sync.dma_start(
                    out=out_d[oc * 128 : (oc + 1) * 128, qc * 512 : (qc + 1) * 512],
                    in_=fsb[:],
                )

            # ---- schedule (issue order = scheduler priority) -------------
            for rc in range(RC):
                v_proj(rc)
            for p in (0, 1):
                for qc in (0, 1):
                    proj_oc(p, qc)       # q of pair p
                    proj_oc(KC + p, qc)  # k of pair p
            attn_group(0, 0)
            for p in (2, 3):
                for qc in (0, 1):
                    proj_oc(p, qc)
                    proj_oc(KC + p, qc)
            attn_group(1, 0)
            for p in (4, 5):
                for qc in (0, 1):
                    proj_oc(p, qc)
                    proj_oc(KC + p, qc)
            attn_group(2, 0)
            attn_group(0, 1)
            for oc in range(KC):
                out_proj(oc, 0)
            attn_group(1, 1)
            attn_group(2, 1)
            for oc in range(KC):
                out_proj(oc, 1)
            if _dbg:
                nc.sync.dma_start(out=dbg_q[:], in_=q_sb[:])
                nc.sync.dma_start(out=dbg_k[:], in_=k_sb[:])
                nc.sync.dma_start(out=dbg_v[:], in_=v_sb[:])
                nc.sync.dma_start(out=dbg_attn[:], in_=attn_sb[:])
                nc.sync.dma_start(out=dbg_recip[:], in_=recip_d[:])

    split_sync_waits(nc, max_waits=1)
    return nc


def _host_prep(x, w_qkv, w_out, b_out):
    bf = ml_dtypes.bfloat16
    inv_freq = 1.0 / (10000.0 ** (np.arange(0, DH, 2, dtype=np.float32) / DH))
    t = np.arange(N, dtype=np.float32)
    freqs = np.outer(t, inv_freq)
    emb = np.concatenate([freqs, freqs], axis=1)        # [N, DH]
    cos2 = np.tile(np.cos(emb).T.astype(np.float32), (2, 1)).astype(bf)
    sin2 = np.tile(np.sin(emb).T.astype(np.float32), (2, 1)).astype(bf)

    perm = np.zeros((128, 128), np.float32)
    for blk in range(2):
        o = blk * 64
        for m in range(32):
            perm[o + m + 32, o + m] = -1.0
        for m in range(32, 64):
            perm[o + m - 32, o + m] = 1.0
    perm = perm.astype(bf)

    xt = np.ascontiguousarray(x.transpose(0, 2, 1)).astype(bf)
    shared = {
        "wq": np.ascontiguousarray(w_qkv).astype(bf),
        "wo": np.ascontiguousarray(w_out).astype(bf),
        "bo": np.ascontiguousarray(b_out).astype(bf),
        "cos2": np.ascontiguousarray(cos2),
        "sin2": np.ascontiguousarray(sin2),
        "perm": np.ascontiguousarray(perm),
    }
    return [dict(shared, xt=np.ascontiguousarray(xt[i])) for i in range(B)]


_NC_CACHE = {}
LAST_EXEC_NS = [None]


def _run(in_maps, trace=False, with_bias=True):
    if with_bias not in _NC_CACHE:
        _NC_CACHE[with_bias] = build_nc(with_bias=with_bias)
    res = run_bass_kernel_spmd(
        _NC_CACHE[with_bias], in_maps, list(range(B)), trace=trace
    )
    LAST_EXEC_NS[0] = res.exec_time_ns
    out_t = np.stack([np.asarray(res.results[i]["out"]) for i in range(B)])
    return np.ascontiguousarray(out_t.transpose(0, 2, 1)).astype(np.float32)


def kernel(x, w_qkv, w_out, b_out, _trace=False):
    b_out = np.asarray(b_out, dtype=np.float32)
    in_maps = _host_prep(
        np.asarray(x, dtype=np.float32),
        np.asarray(w_qkv, dtype=np.float32),
        np.asarray(w_out, dtype=np.float32),
        b_out,
    )
    return _run(in_maps, trace=_trace, with_bias=bool(np.any(b_out)))
